# revision 16
# baseline (speedup 1.0000x reference)
"""Trainium2 Bass kernel for nn_CCInitPi (vq_codebook).

Reference computation (D=128, N=8192, K=256):
    AX[d,n,c]  = sum_e N_A[d,e,c] * X[e,n]
    Amu[d,c]   = sum_e N_A[d,e,c] * N_mu[e,c]
    sq[n,c]    = sum_d (AX[d,n,c] - Amu[d,c])^2
    Pi         = softmax(gamma*sq, axis=c).T          # (K, N)
    out        = vstack(X, Pi)                        # (D+K, N)

Strategy: shard N across the 8 cores (1024 columns each); N_A/N_mu are
replicated.  Per core, for each codebook entry c the tensor engine computes
Z_c = A_c^T X (psum layout: d on partitions, n on free); the scalar engine
forms (Z - Amu_c)^2 directly (Amu_c rides the activation's per-partition
bias) while the vector engine's share is computed as (Z - 2*Amu_c)*Z in a
single fused scalar_tensor_tensor pass (the missing +Amu^2 term is added
into the PSUM accumulator with a rank-1 matmul).  A second matmul against a
sliding ones-column window reduces over the d partitions, accumulating
sq^T(c-rows, n-cols) in PSUM — issued as 4 concurrent 32-row col-tiles
(tile_position) so four codebook entries share one pass through the array.
Softmax over c runs after two PE transposes put (n, c) tiles on the
partitions; gamma rides the activation's per-partition scale and the exp's
accum_out gives the softmax denominator for free.  Matmul inputs are fp16
(same PE speed as bf16, 4x faster than fp32); all accumulation is fp32 in
PSUM.
"""

import os
from contextlib import ExitStack

import numpy as np

import concourse.bass as bass
import concourse.mybir as mybir
import concourse.tile as tile
from concourse.bass_utils import run_bass_kernel_spmd

dt = mybir.dt
F16 = dt.float16
F32 = dt.float32
AF = mybir.ActivationFunctionType
ALU = mybir.AluOpType

D, N, K = 128, 8192, 256
NCORES = 8
NLOC = N // NCORES  # 1024 columns of X per core
LAG = 2  # software-pipeline distance between Z production and d-reduction

# test.py hooks
TRACE = bool(int(os.environ.get("KERNEL_TRACE", "0")))
LAST_RESULTS = None

_CACHE = {}


def _act_share(j, b):
    """Which engine squares entry (j, b): True -> ScalarE, False -> VectorE.
    ~63% on ScalarE balances the one-pass ScalarE square (1.336 cyc/elem
    @1.2GHz) against the two-pass VectorE path (1.847 cyc/elem @0.96GHz)."""
    return b < 2 or (b == 2 and j % 2 == 0)


def _c_of(half, b, j):
    return half * 128 + b * 32 + j


def _chunk_order(ach):
    """A-chunk DMA order matching first use: per j-group the four b-blocks,
    so chunks interleave with stride 4 within each half."""
    per_block = 32 // ach  # j-groups per 32-entry block
    order = []
    for half in range(2):
        for jg in range(per_block):
            for b in range(4):
                order.append(half * 16 + b * per_block + jg)
    return order


def _split_mm_waits(nc):
    """Walrus' engine-instruction formats carry a single sem-wait slot on
    this target.  Tile's sem-assigner can leave >1 wait on an instruction
    (no transitive vector-clock tracking); hoist the extras onto chained
    same-engine NoOps placed immediately before — each engine executes its
    stream in order, so semantics are unchanged."""
    k = 0
    for f in nc.m.functions:
        for bb in f.blocks:
            new = []
            changed = False
            for ins in bb.instructions:
                si = ins.sync_info
                if si is not None and len(si.on_wait) > 1:
                    waits = list(si.on_wait)
                    for w in waits[:-1]:
                        nop = mybir.InstNoOp(name=f"I-wsplit-{k}")
                        k += 1
                        nop.engine = ins.engine
                        nop.sync_info = mybir.SyncInfo(on_wait=[w], on_update=[])
                        new.append(nop)
                    ins.sync_info = mybir.SyncInfo(
                        on_wait=[waits[-1]], on_update=list(si.on_update)
                    )
                    changed = True
                new.append(ins)
            if changed:
                bb.instructions = new
    return k


def _build(n_loc=NLOC, split_waits=True, reps=1):
    """Build the per-core Bass module (identical program on all cores)."""
    nc = bass.Bass("TRN2", debug=False)

    a_d = nc.dram_tensor("a", [D, K * D], F16, kind="ExternalInput").ap()
    x_d = nc.dram_tensor("x", [D, n_loc], F16, kind="ExternalInput").ap()
    mu_d = nc.dram_tensor("mu", [D, K], F16, kind="ExternalInput").ap()
    g_d = nc.dram_tensor("g", [D, 1], F32, kind="ExternalInput").ap()
    id_d = nc.dram_tensor("ident", [D, D], F16, kind="ExternalInput").ap()
    w_d = nc.dram_tensor("w", [D, 2 * D], F16, kind="ExternalInput").ap()
    w32_d = nc.dram_tensor("w32", [D, 64], F16, kind="ExternalInput").ap()
    pi_d = nc.dram_tensor("pi", [K, n_loc], F16, kind="ExternalOutput").ap()

    ACH = 8  # codebook entries per A-chunk DMA
    NCH = K // ACH
    pieces = [(s, min(512, n_loc - s)) for s in range(0, n_loc, 512)]
    ntiles = n_loc // D

    with tile.TileContext(nc) as tc:
        with ExitStack() as ctx:
            consts = ctx.enter_context(tc.tile_pool(name="consts", bufs=1))
            apool = ctx.enter_context(tc.tile_pool(name="apool", bufs=1))
            zsqp = ctx.enter_context(tc.tile_pool(name="zsqp", bufs=4 * (LAG + 1) + 1))
            tdp = ctx.enter_context(tc.tile_pool(name="tdp", bufs=3))
            ep = ctx.enter_context(tc.tile_pool(name="ep", bufs=2))
            sp = ctx.enter_context(tc.tile_pool(name="sp", bufs=2))
            sqp = ctx.enter_context(tc.tile_pool(name="sqp", bufs=3))
            pop = ctx.enter_context(tc.tile_pool(name="pop", bufs=2))

            # ---- load constants / inputs ----
            x_t = consts.tile([D, n_loc], F16, name="x_t")
            nc.sync.dma_start(x_t[:], x_d[:])
            mu_t = consts.tile([D, K], F16, name="mu_t")
            nc.sync.dma_start(mu_t[:], mu_d[:])
            g_t = consts.tile([D, 1], F32, name="g_t")
            nc.sync.dma_start(g_t[:], g_d[:])
            id_t = consts.tile([D, D], F16, name="id_t")
            nc.sync.dma_start(id_t[:], id_d[:])
            w_t = consts.tile([D, 2 * D], F16, name="w_t")
            nc.sync.dma_start(w_t[:], w_d[:])
            w32_t = consts.tile([D, 64], F16, name="w32_t")
            nc.sync.dma_start(w32_t[:], w32_d[:])
            a_t = [None] * NCH
            for i in _chunk_order(ACH):
                at = apool.tile([D, ACH * D], F16, name=f"a_t{i}", tag=f"a{i}")
                nc.sync.dma_start(at[:], a_d[:, i * ACH * D : (i + 1) * ACH * D])
                a_t[i] = at

            def a_sl(c):
                return a_t[c // ACH][:, (c % ACH) * D : (c % ACH + 1) * D]

            for rep in range(reps):
                # ---- prologue: Amu[:, c] = A_c^T mu_c; derive -Amu, 2*Amu,
                # and smask[c] = mask[c] * sum_d Amu[d,c]^2 ----
                namu = consts.tile([D, K], F32, name=f"namu_{rep}", tag="namu", bufs=2)
                with tc.tile_pool(name="amup", bufs=1, space="PSUM") as amup:
                    pamu = amup.tile([D, K], F32, name=f"pamu_{rep}")
                    for ch in _chunk_order(ACH):
                        for cc in range(ACH):
                            c = ch * ACH + cc
                            nc.tensor.matmul(
                                pamu[:, c : c + 1], a_sl(c), mu_t[:, c : c + 1],
                                start=True, stop=True,
                            )
                    nc.vector.tensor_scalar_mul(namu[:], pamu[:], -1.0)

                # ---- main: Z, (Z-Amu)^2, and the 4-way col-tiled d-reduce ----
                sq16 = []
                main_ctx = ExitStack()
                zp = main_ctx.enter_context(
                    tc.tile_pool(name="zp", bufs=6, space="PSUM")
                )
                qp = main_ctx.enter_context(
                    tc.tile_pool(name="qp", bufs=2, space="PSUM")
                )
                for half in range(2):
                    sqh = sqp.tile(
                        [D, n_loc], F16, name=f"sq16_{rep}_{half}", tag="sq16"
                    )
                    for pi_, (s, cs) in enumerate(pieces):
                        qt = qp.tile(
                            [D, cs], F32, name=f"q_{rep}_{half}_{pi_}", tag="q"
                        )
                        # seed the bank: zero-write (W cols 0..127 are zero)
                        # opens the accumulation group + sets has_written
                        nc.tensor.matmul(
                            qt[:], w_t[:, 0:D], x_t[:, s : s + cs],
                            start=True, stop=False,
                        )
                        zq = {}

                        def emit_q(jq, last):
                            for b in range(4):
                                nc.tensor.matmul(
                                    qt[32 * b : 32 * b + 32, :],
                                    w32_t[:, 32 - jq : 64 - jq],
                                    zq.pop((jq, b))[:],
                                    start=False,
                                    stop=False,
                                    tile_position=(0, 32 * b),
                                    skip_group_check=True,
                                )

                        for j in range(32):
                            for b in range(4):
                                c = _c_of(half, b, j)
                                zt = zp.tile(
                                    [D, cs], F32, name=f"z_{rep}_{c}_{pi_}", tag="z"
                                )
                                nc.tensor.matmul(
                                    zt[:], a_sl(c), x_t[:, s : s + cs],
                                    start=True, stop=True,
                                )
                                z2 = zsqp.tile(
                                    [D, cs], F16, name=f"zsq_{rep}_{c}_{pi_}",
                                    tag="zsq",
                                )
                                if _act_share(j, b):
                                    nc.scalar.activation(
                                        z2[:], zt[:], AF.Square,
                                        bias=namu[:, c : c + 1], scale=1.0,
                                    )
                                else:
                                    td = tdp.tile(
                                        [D, cs], F16, name=f"td_{rep}_{c}_{pi_}",
                                        tag="td",
                                    )
                                    nc.vector.tensor_scalar_add(
                                        td[:], zt[:], namu[:, c : c + 1]
                                    )
                                    nc.vector.tensor_mul(z2[:], td[:], td[:])
                                zq[(j, b)] = z2
                            if j >= LAG:
                                emit_q(j - LAG, last=(j - LAG == 31))
                        for j in range(32 - LAG, 32):
                            emit_q(j, last=(j == 31))
                        # close the accumulation group across all partitions
                        nc.tensor.matmul(
                            qt[:], w_t[:, 0:D], x_t[:, s : s + cs],
                            start=False, stop=True,
                        )
                        nc.vector.tensor_copy(sqh[:, s : s + cs], qt[:])
                    sq16.append(sqh)
                main_ctx.close()

                # ---- softmax over c (per 128-column tile of n) ----
                post_ctx = ExitStack()
                tp = post_ctx.enter_context(
                    tc.tile_pool(name="tp", bufs=2, space="PSUM")
                )
                pp_pool = post_ctx.enter_context(
                    tc.tile_pool(name="ppp", bufs=2, space="PSUM")
                )
                pp = [
                    pp_pool.tile([D, n_loc], F16, name=f"pp_{rep}_{h}", tag=f"pp{h}")
                    for h in range(2)
                ]
                for t in range(ntiles):
                    pt = tp.tile([D, K], F16, name=f"pt_{rep}_{t}", tag="pt")
                    for half in range(2):
                        nc.tensor.transpose(
                            pt[:, half * D : (half + 1) * D],
                            sq16[half][:, t * D : (t + 1) * D],
                            id_t[:],
                        )
                    e16 = ep.tile([D, K], F16, name=f"e16_{rep}_{t}", tag="e16")
                    s_ = sp.tile([D, 1], F32, name=f"s_{rep}_{t}", tag="s")
                    nc.scalar.activation(
                        e16[:], pt[:], AF.Exp, bias=0.0, scale=g_t[:, 0:1],
                        accum_out=s_[:],
                    )
                    r_ = sp.tile([D, 1], F32, name=f"r_{rep}_{t}", tag="r")
                    nc.vector.reciprocal(r_[:], s_[:])
                    p16 = ep.tile([D, K], F16, name=f"p16_{rep}_{t}", tag="p16")
                    nc.vector.tensor_scalar_mul(p16[:], e16[:], r_[:, 0:1])
                    for half in range(2):
                        nc.tensor.transpose(
                            pp[half][:, t * D : (t + 1) * D],
                            p16[:, half * D : (half + 1) * D],
                            id_t[:],
                        )

                # ---- write Pi back ----
                for half in range(2):
                    po = pop.tile([D, n_loc], F16, name=f"po_{rep}_{half}", tag="po")
                    nc.vector.tensor_copy(po[:], pp[half][:])
                    nc.sync.dma_start(pi_d[half * D : (half + 1) * D, :], po[:])
                post_ctx.close()

    if split_waits:
        _split_mm_waits(nc)
    return nc


def _get_module(n_loc=NLOC):
    if n_loc not in _CACHE:
        _CACHE[n_loc] = _build(n_loc)
    return _CACHE[n_loc]


def make_in_maps(X, N_A, N_mu, gamma, n_cores=NCORES, n_loc=NLOC):
    """Host-side layout prep + sharding (N split across cores)."""
    X = np.asarray(X, dtype=np.float32)
    N_A = np.asarray(N_A, dtype=np.float32)
    N_mu = np.asarray(N_mu, dtype=np.float32)
    gamma = np.float32(np.asarray(gamma))

    # A_host[e, c, d] = N_A[d, e, c]
    a_flat = np.ascontiguousarray(np.transpose(N_A, (1, 2, 0))).astype(np.float16)
    a_flat = a_flat.reshape(D, K * D)
    x16 = X.astype(np.float16)
    mu16 = N_mu.astype(np.float16)
    g = np.full((D, 1), gamma, dtype=np.float32)
    ident = np.eye(D, dtype=np.float16)
    w = np.zeros((D, 2 * D), np.float16)
    w[:, D] = 1.0
    w32 = np.zeros((D, 64), np.float16)
    w32[:, 32] = 1.0
    in_maps = []
    for i in range(n_cores):
        in_maps.append(
            {
                "a": a_flat,
                "x": np.ascontiguousarray(x16[:, i * n_loc : (i + 1) * n_loc]),
                "mu": mu16,
                "g": g,
                "ident": ident,
                "w": w,
                "w32": w32,
            }
        )
    return in_maps


def kernel(X, N_A, N_mu, gamma):
    global LAST_RESULTS
    X = np.asarray(X, dtype=np.float32)
    nc = _get_module()
    in_maps = make_in_maps(X, N_A, N_mu, gamma)
    res = run_bass_kernel_spmd(nc, in_maps, list(range(NCORES)), trace=TRACE)
    LAST_RESULTS = res
    pi = np.concatenate(
        [res.results[i]["pi"].astype(np.float32) for i in range(NCORES)], axis=1
    )
    return np.concatenate([X, pi], axis=0)
